# revision 24
# baseline (speedup 1.0000x reference)
"""Trainium2 Bass kernel for nn_MultiHeadAttn (16-head attention + out-proj +
residual + layernorm), distributed over 8 NeuronCores.

Sharding: core c handles batch b = c//2 and query rows [512*(c%2), 512*(c%2)+512).
Each core recomputes the full K/V projections for its batch (duplicated between
the two cores of a batch) so there are no collectives; every core is fully
independent and the host just concatenates the 8 output slabs.

All heavy matmuls run fp8e4m3 with DoubleRow (0.5 cycles/row on the PE):
  qhT/khT    = (q|k @ w)          fp8 DR over d_model 128-chunk pairs
  qhT32/khT32: fp8 copies reshuffled via SBUF->SBUF DMA into a [32, 2, *]
               layout so the dk=64 contraction of QK can also run DR
  scoresT    = khT32.T-chunks @ qhT32   fp8 DR, [key_chunk=128, 512] PSUM
  e          = exp(scoresT / 32), fp8: most tiles on ACT (spline exp, fp8
               out); a tunable subset on DVE via the int8-Schraudolph bit
               trick (one tensor_scalar: bits8 = round(8*log2e/32 * s +
               55.66) written through an int8 bitcast of the fp8 tile; max
               rel err ~7.5% vs fp8-quantized true exp's ~5.9%, rms ~3.1%
               vs 2.7%) to offload the ACT bottleneck
  vh_ext     = v @ w_v staged as [128, pair, kc, 192]: cols 0:64 = head0,
               64:128 = zeros (memset once), 128:192 = head1
  OT/Z       : ALL DoubleRow. head1 writes rows 64-127 through an M=128
               matmul whose lhsT is [zeros(64) | v1(64)] -- the zero half
               accumulates +0 into head0's rows, so the dst partition
               offset stays 0 (walrus s3d3_mm_valid_dst_partition forbids
               DR at offset 64). Per (kc2, hh): one OT + one Z matmul at
               256 PE cycles each vs the old plain-fp8 head1 path's 2048.
               Z uses the same trick with [ones|zeros|ones] lhsT slices;
               accumulation order per PSUM bank starts with the hh=1 M=128
               matmul (start=True resets all 128 rows).
  OTn        = OT * (1/Z)         DVE reciprocal + scalar_tensor_tensor
  out        = OTn.T @ w_projT    fp8 DR over head-pair pairs
  final      = layernorm(out + q_resid): mean/var via bn_stats; rsqrt via
               a quake-style seed (one DVE tensor_scalar in int32 bitcast
               domain: bits = round(C - 0.5*bits(var))) plus one Newton
               step with sqrt((D-1)/D) folded into its constants. No ACT
               Ln/Exp -> the whole kernel uses exactly one ACT table set
               (exp_and_others), eliminating two ~2.7us table reloads per
               iteration.

The attention inner loop interleaves Q/K/V projections for pair p+1 into
pair p's exp-wait gaps; PV/Z matmuls lag their exp by two steps (one near
the pair tail). Cross-repeat software pipelining: input tiles are double-
buffered so the next repeat's DMA loads fire at pair 1, the next repeat's
pair-0 projections hide in pair 7's gaps, and the epilogue is split into
four per-qc chains (out-proj matmuls, bn stats, rsqrt, ACT normalize,
store) drained into pairs 0-3 of the NEXT repeat (otn double-buffered to
decouple the out-proj reads from the next repeat's stt writes).

Numerics: validated in np emulation against the reference: all-fp8 with
40% Schraudolph-exp tiles gives rel err ~2.1e-3 (budget 2e-2); measured
on HW: 2.5e-3.
"""

import sys

sys.path.insert(0, "/opt/trn_rl_repo")

import numpy as np
import ml_dtypes

import concourse.bass as bass
import concourse.mybir as mybir
import concourse.tile as tile
from concourse import bacc
from concourse.bass_utils import run_bass_kernel_spmd

D = 1024          # d_model
H = 16            # heads
DK = 64           # head dim
L = 1024          # seq len (keys)
Q = 512           # query rows per core
P = 128
KC = D // P       # 8 contraction chunks of 128
PAIRS = H // 2    # 8 head pairs
QCN = Q // P      # 4 query chunks
EPS = 1e-5
TEMP_INV = 1.0 / 32.0  # 1/sqrt(d_model)

# int8-Schraudolph exp constants: bits8 = round(A_TR * s + B_TR) viewed fp8e4m3
LOG2E = float(np.log2(np.e))
A_TR = 8.0 * LOG2E / 32.0
B_TR = 55.66
# quake rsqrt seed: bits32 = round(RSQRT_C - 0.5 * bits32(x)); 1 Newton step
RSQRT_C = 1597463174.0           # 0x5f375a86
LN_C = float(np.sqrt((D - 1.0) / D))  # folds unbiased-var factor into Newton

BF = mybir.dt.bfloat16
F8 = mybir.dt.float8e4
F32 = mybir.dt.float32
I8 = mybir.dt.int8
I32 = mybir.dt.int32
AF = mybir.ActivationFunctionType
ALU = mybir.AluOpType
DR = mybir.MatmulPerfMode.DoubleRow
BF_NP = ml_dtypes.bfloat16
F8_NP = ml_dtypes.float8_e4m3

# which (kc2, hh) slots of every pair compute exp on the DVE instead of ACT,
# plus extra per-pair slots for fine-grained ACT/DVE balance
DVE_EXP_SLOTS = frozenset({(1, 1)})
DVE_EXP_EXTRA = frozenset({(p, 3, 1) for p in (0, 2, 4, 6)} | {(1, 2, 1)})

_CACHE: dict = {}


def _build(trivial_ln: bool, repeat: int = 1, dve_slots=DVE_EXP_SLOTS,
           dve_extra=DVE_EXP_EXTRA):
    nc = bacc.Bacc(None, target_bir_lowering=False)

    qT = nc.dram_tensor("qT", [D, Q], F8, kind="ExternalInput")
    kT = nc.dram_tensor("kT", [D, L], F8, kind="ExternalInput")
    vT = nc.dram_tensor("vT", [D, L], F8, kind="ExternalInput")
    wq = nc.dram_tensor("wq", [D, H * DK], F8, kind="ExternalInput")
    wk = nc.dram_tensor("wk", [D, H * DK], F8, kind="ExternalInput")
    wv = nc.dram_tensor("wv", [D, H * DK], F8, kind="ExternalInput")
    wp = nc.dram_tensor("wp", [H * DK, D], F8, kind="ExternalInput")
    qres = nc.dram_tensor("qres", [Q, D], BF, kind="ExternalInput")
    ident = nc.dram_tensor("ident", [P, P], BF, kind="ExternalInput")
    lnsc = nc.dram_tensor("lnsc", [D], F32, kind="ExternalInput")
    lnof = nc.dram_tensor("lnof", [D], F32, kind="ExternalInput")
    out = nc.dram_tensor("out", [Q, D], F32, kind="ExternalOutput")

    with tile.TileContext(nc) as tc:
        with (
            tc.tile_pool(name="consts", bufs=1) as consts,
            tc.tile_pool(name="sexp", bufs=12) as sexp,
            tc.tile_pool(name="znorm", bufs=3) as znorm,
            tc.tile_pool(name="lnp", bufs=3) as lnp,
            tc.tile_pool(name="psA", bufs=3, space="PSUM") as psA,
            tc.tile_pool(name="psOT", bufs=1, space="PSUM") as psOT,
            tc.tile_pool(name="psZ", bufs=1, space="PSUM") as psZ,
            # psA 3x[128,1024]f32 = 6 banks; ot/z 1 bank each -> 8 total
        ):
            # -------- constants shared across repeats --------
            # Z lhsT [128, 2, 192]: cols 0:64 ones (z0, M=64), 64:128 zeros +
            # 128:192 ones (z1 via [zeros|ones] M=128 at dst offset 0)
            zo_sb = consts.tile([P, 2, 192], F8, tag="zo")
            nc.vector.memset(zo_sb[:], 1.0)
            nc.vector.memset(zo_sb[:, :, 64:128], 0.0)
            ident_sb = consts.tile([P, P], BF, tag="ident")
            nc.sync.dma_start(ident_sb[:], ident.ap())
            if not trivial_ln:
                lnsc_b = consts.tile([P, D], F32, tag="lnsc")
                nc.gpsimd.dma_start(
                    lnsc_b[:],
                    bass.AP(tensor=lnsc.ap().tensor, offset=0, ap=[[0, P], [1, D]]),
                )
                lnof_b = consts.tile([P, D], F32, tag="lnof")
                nc.gpsimd.dma_start(
                    lnof_b[:],
                    bass.AP(tensor=lnof.ap().tensor, offset=0, ap=[[0, P], [1, D]]),
                )
            # magic-seed constant tile for the DVE epilogue (int32 bits)
            rsq_c = lnp.tile([P, 2], F32, tag="rsqc")
            nc.vector.memset(rsq_c[:].bitcast(I32), 0x5F375A86)

            # per-repeat tile state. Input tiles are DOUBLE-BUFFERED (tag
            # alternates with the repeat parity) so the next repeat's DMA
            # loads can issue early in the current repeat with no WAR wait;
            # rep-local intermediates (qhT8/khT32/vh/otn/...) stay
            # single-buffered -- cross-repeat overlap only touches disjoint
            # sub-tile regions, which the framework tracks.
            vhz_zeroed = [False]

            def emit_loads(parity):
                b = parity & 1
                t = {}
                t["qT_sb"] = consts.tile([P, KC, Q], F8, tag=f"qT{b}", name="qT_sb")
                nc.sync.dma_start(t["qT_sb"][:], qT.ap().rearrange("(c p) q -> p c q", p=P))
                t["wq_sb"] = consts.tile([P, KC, H * DK], F8, tag=f"wq{b}", name="wq_sb")
                nc.sync.dma_start(t["wq_sb"][:], wq.ap().rearrange("(c p) m -> p c m", p=P))
                t["kT_sb"] = consts.tile([P, KC, L], F8, tag=f"kT{b}", name="kT_sb")
                nc.sync.dma_start(t["kT_sb"][:], kT.ap().rearrange("(c p) q -> p c q", p=P))
                t["wk_sb"] = consts.tile([P, KC, H * DK], F8, tag=f"wk{b}", name="wk_sb")
                nc.sync.dma_start(t["wk_sb"][:], wk.ap().rearrange("(c p) m -> p c m", p=P))
                t["vT_sb"] = consts.tile([P, KC, L], F8, tag=f"vT{b}", name="vT_sb")
                nc.gpsimd.dma_start(
                    t["vT_sb"][:], vT.ap().rearrange("(c p) q -> p c q", p=P))
                t["wv_sb"] = consts.tile([P, KC, H * DK], F8, tag=f"wv{b}", name="wv_sb")
                nc.gpsimd.dma_start(
                    t["wv_sb"][:], wv.ap().rearrange("(c p) m -> p c m", p=P))
                t["wp_sb"] = consts.tile([P, PAIRS, D], F8, tag=f"wp{b}", name="wp_sb")
                nc.gpsimd.dma_start(
                    t["wp_sb"][:], wp.ap().rearrange("(c p) m -> p c m", p=P))
                t["qres_sb"] = consts.tile([P, QCN, D], BF, tag=f"qres{b}", name="qres_sb")
                nc.gpsimd.dma_start(
                    t["qres_sb"][:], qres.ap().rearrange("(c p) d -> p c d", p=P))

                t["qhT8"] = consts.tile([P, PAIRS, Q], F8, tag="qhT8", name="qhT8")
                t["khT8"] = consts.tile([P, PAIRS, L], F8, tag="khT8", name="khT8")
                # [32,2] DR layouts: head h lives on partitions 32*(h%4)+[0,32),
                # free dims (h//4, j, seq) with dk = 32*j + p
                t["qhT32"] = consts.tile([P, 4, 2, Q], F8, tag="qhT32", name="qhT32")
                t["khT32"] = consts.tile([P, 4, 2, L], F8, tag="khT32", name="khT32")
                # vh_ext per pair: [P, pair, kc, 3 blocks, 64]: block 0 head0
                # v, block 1 zeros (for the M=128 [zeros|v1] PV trick), block
                # 2 head1 v. Zeros memset once (tag-stable buffer); the
                # conversion writes blocks 0 and 2 in ONE strided-dst copy.
                t["vh"] = consts.tile([P, PAIRS, KC, 3, 64], F8, tag="vh", name="vh")
                # otn double-buffered: the epilogue of rep N reads otn while
                # rep N+1's stt writes the other buffer
                t["otn"] = consts.tile([P, PAIRS, Q], F8, tag=f"otn{b}", name="otn")
                if not vhz_zeroed[0]:
                    nc.vector.memset(t["vh"][:, :, :, 1, :], 0.0)
                    vhz_zeroed[0] = True
                return t

            # ---------------- per-pair projection emitters ----------------
            def emit_qkproj(m, cur):
                """Q+K projections for pair m (against state `cur`) as thunks."""
                st = {}

                def qmm(c2):
                    if "q" not in st:
                        st["q"] = psA.tile([P, 2 * Q], F32, tag="mm", name=f"psq_{m}")
                    nc.tensor.matmul(
                        st["q"][:, :Q],
                        cur["wq_sb"][:, 2 * c2 : 2 * c2 + 2, m * P : (m + 1) * P],
                        cur["qT_sb"][:, 2 * c2 : 2 * c2 + 2, :],
                        start=(c2 == 0), stop=(c2 == KC // 2 - 1),
                        perf_mode=DR,
                    )

                def kmm(half, c2):
                    if "k" not in st:
                        st["k"] = psA.tile([P, L], F32, tag="mm", name=f"psk_{m}")
                    nc.tensor.matmul(
                        st["k"][:, half * 512 : (half + 1) * 512],
                        cur["wk_sb"][:, 2 * c2 : 2 * c2 + 2, m * P : (m + 1) * P],
                        cur["kT_sb"][:, 2 * c2 : 2 * c2 + 2, half * 512 : (half + 1) * 512],
                        start=(c2 == 0), stop=(c2 == KC // 2 - 1),
                        perf_mode=DR,
                    )

                def shuffle(is_q, hh, j):
                    # split across the Pool (SWDGE) and SP (HWDGE) rings
                    h = 2 * m + hh
                    dst32 = cur["qhT32"] if is_q else cur["khT32"]
                    src8 = cur["qhT8"] if is_q else cur["khT8"]
                    eng = nc.gpsimd if is_q else nc.sync
                    eng.dma_start(
                        dst32[32 * (h % 4) : 32 * (h % 4) + 32, h // 4, j, :],
                        src8[hh * DK + 32 * j : hh * DK + 32 * j + 32, m, :],
                    )

                th = []
                for c2 in range(KC // 2):
                    th.append(lambda c2=c2: qmm(c2))
                th.append(lambda: nc.vector.tensor_copy(
                    cur["qhT8"][:, m, :], st["q"][:, :Q]))
                for hh in range(2):
                    for j in range(2):
                        th.append(lambda hh=hh, j=j: shuffle(True, hh, j))
                for half in range(2):
                    for c2 in range(KC // 2):
                        th.append(lambda half=half, c2=c2: kmm(half, c2))
                th.append(lambda: nc.vector.tensor_copy(
                    cur["khT8"][:, m, :], st["k"][:]))
                for hh in range(2):
                    for j in range(2):
                        th.append(lambda hh=hh, j=j: shuffle(False, hh, j))
                return th

            def emit_vproj(m, cur):
                """V projection for pair m into the vh_ext [.., 192] layout."""
                st = {}

                def vmm(kc, c2):
                    if "v" not in st:
                        st["v"] = psA.tile([P, KC, P], F32, tag="mm", name=f"psv_{m}")
                    nc.tensor.matmul(
                        st["v"][:, kc, :],
                        cur["vT_sb"][:, 2 * c2 : 2 * c2 + 2, kc * P : (kc + 1) * P],
                        cur["wv_sb"][:, 2 * c2 : 2 * c2 + 2, m * P : (m + 1) * P],
                        start=(c2 == 0), stop=(c2 == KC // 2 - 1),
                        perf_mode=DR,
                    )

                th = []
                for kc in range(KC):
                    for c2 in range(KC // 2):
                        th.append(lambda kc=kc, c2=c2: vmm(kc, c2))
                # one strided-dst copy fills head0 (block 0) and head1
                # (block 2), skipping the zeros block
                th.append(lambda: nc.vector.tensor_copy(
                    cur["vh"][:, m, :, 0:3:2, :],
                    st["v"][:].rearrange("p k (g c) -> p k g c", g=2)))
                return th

            def run_thunks(th):
                for t in th:
                    t()

            def emit_epilogue_chains(cur):
                """Output projection + residual + layernorm for the rep whose
                state is `cur`, as FOUR independent per-qc thunk chains. Each
                chain is drained into one pair of the NEXT repeat's attention
                (or run inline after the last repeat): out-proj matmuls fill
                PE gaps, bn/rsqrt the DVE, and the final normalize runs on
                ACT (Identity with per-partition scale/bias) between exps."""
                otn_t, wp_t, qres_t = cur["otn"], cur["wp_sb"], cur["qres_sb"]

                def make_chain(qc):
                    st = {}

                    def fpmm(half, p2, qc=qc, st=st):
                        if "fp" not in st:
                            st["fp"] = psA.tile([P, D], F32, tag="mm", name=f"fp_{qc}")
                        nc.tensor.matmul(
                            st["fp"][:, half * 512 : (half + 1) * 512],
                            otn_t[:, 2 * p2 : 2 * p2 + 2, qc * P : (qc + 1) * P],
                            wp_t[:, 2 * p2 : 2 * p2 + 2, half * 512 : (half + 1) * 512],
                            start=(p2 == 0), stop=False,
                            perf_mode=DR,
                        )

                    def identmm(half, qc=qc, st=st):
                        # residual folded into the accumulation
                        nc.tensor.matmul(
                            st["fp"][:, half * 512 : (half + 1) * 512],
                            ident_sb[:],
                            qres_t[:, qc, half * 512 : (half + 1) * 512],
                            start=False, stop=True,
                        )

                    def bn(j, st=st):
                        if "stats" not in st:
                            st["stats"] = lnp.tile([P, 2, 6], F32, tag="stats", name="stats")
                        nc.vector.bn_stats(st["stats"][:, j, :],
                                           st["fp"][:, j * 512 : (j + 1) * 512])

                    def aggr(st=st):
                        st["mv"] = lnp.tile([P, 2], F32, tag="mv", name="mv")
                        nc.vector.bn_aggr(st["mv"][:], st["stats"][:])

                    def rchain(st=st):
                        # rinv = LN_C/sqrt(var): quake seed + one Newton step,
                        # then negms = -mean*rinv, all tiny [P,1] DVE ops
                        mv = st["mv"]
                        y0 = lnp.tile([P, 1], F32, tag="y0")
                        nc.vector.scalar_tensor_tensor(
                            y0[:].bitcast(I32), mv[:, 1:2].bitcast(I32), -0.5,
                            rsq_c[:, 0:1].bitcast(I32), ALU.mult, ALU.add,
                        )
                        ta = lnp.tile([P, 1], F32, tag="ta")
                        nc.vector.tensor_mul(ta[:], y0[:], mv[:, 1:2])
                        t2 = lnp.tile([P, 1], F32, tag="t2")
                        nc.vector.tensor_mul(t2[:], ta[:], y0[:])
                        t3 = lnp.tile([P, 1], F32, tag="t3")
                        nc.vector.tensor_scalar(
                            t3[:], t2[:], -0.5 * LN_C, 1.5 * LN_C, ALU.mult, ALU.add
                        )
                        rinv = lnp.tile([P, 1], F32, tag="rinv")
                        nc.vector.tensor_mul(rinv[:], y0[:], t3[:])
                        negms = lnp.tile([P, 1], F32, tag="negms")
                        nc.vector.scalar_tensor_tensor(
                            negms[:], mv[:, 0:1], -1.0, rinv[:], ALU.mult, ALU.mult
                        )
                        st["rinv"], st["negms"] = rinv, negms

                    def final(qc=qc, st=st):
                        o_sb = lnp.tile([P, D], F32, tag="o", name=f"o_{qc}")
                        nc.scalar.activation(
                            o_sb[:], st["fp"][:], AF.Identity,
                            bias=st["negms"][:, 0:1], scale=st["rinv"][:, 0:1],
                        )
                        if not trivial_ln:
                            nc.vector.tensor_mul(o_sb[:], o_sb[:], lnsc_b[:])
                            nc.vector.tensor_add(o_sb[:], o_sb[:], lnof_b[:])
                        nc.sync.dma_start(out.ap()[qc * P : (qc + 1) * P, :], o_sb[:])

                    th = []
                    for half in range(2):
                        for p2 in range(PAIRS // 2):
                            th.append(lambda half=half, p2=p2: fpmm(half, p2))
                        th.append(lambda half=half: identmm(half))
                    th.append(lambda: bn(0))
                    th.append(lambda: bn(1))
                    th.append(aggr)
                    th.append(rchain)
                    th.append(final)
                    return th

                return [make_chain(qc) for qc in range(QCN)]

            cur = emit_loads(0)
            run_thunks(emit_qkproj(0, cur))
            run_thunks(emit_vproj(0, cur))
            epi_chains: list = []  # previous rep's epilogue, one chain per qc
            for _rep in range(repeat):
                nxt_state = None
                qhT32, khT32 = cur["qhT32"], cur["khT32"]
                vh, otn = cur["vh"], cur["otn"]

                # ---------------- attention (per head pair), pipelined --------
                for p in range(PAIRS):
                    if p == 1 and _rep + 1 < repeat:
                        # kick off the NEXT repeat's input DMAs now -- the
                        # double-buffered tiles have no WAR, so the loads
                        # complete long before pair-7's prologue thunks
                        # consume them
                        nxt_state = emit_loads(_rep + 1)
                    if p + 1 < PAIRS:
                        interleave = emit_qkproj(p + 1, cur) + emit_vproj(p + 1, cur)
                    elif nxt_state is not None:
                        # hoist the NEXT repeat's pair-0 projections into the
                        # LAST pair's exp-wait gaps, so the next attention
                        # phase is ready the moment this repeat ends
                        interleave = (emit_qkproj(0, nxt_state)
                                      + emit_vproj(0, nxt_state))
                    else:
                        interleave = []
                    if p < len(epi_chains):
                        # drain the PREVIOUS repeat's epilogue chain for qc=p
                        # into this pair's exp-wait gaps
                        interleave = epi_chains[p] + interleave
                    ii = 0
                    ot_ps = psOT.tile([P, Q], F32, tag="ot")
                    z_ps = psZ.tile([P, Q], F32, tag="z")
                    pending = []  # lagged PV/Z emissions: (e_tile, kc2, hh)

                    def flush_one():
                        e, kc2, hh = pending.pop(0)
                        first = kc2 == 0
                        last = kc2 == KC // 2 - 1
                        if hh == 1:
                            # M=128 [zeros(64)|v1(64)] lhsT: head1 lands on
                            # rows 64-127 while rows 0-63 accumulate +0, so
                            # the DR dst offset stays 0. start=True on the
                            # kc2==0 instance resets the whole bank (it is
                            # flushed first; see the pending swap below).
                            nc.tensor.matmul(
                                ot_ps[:, :],
                                vh[:, p, 2 * kc2 : 2 * kc2 + 2, 1:3, :],
                                e[:],
                                start=first, stop=last,
                                perf_mode=DR, tile_position=(0, 0),
                            )
                            nc.tensor.matmul(
                                z_ps[:, :],
                                zo_sb[:, :, 64:192],
                                e[:],
                                start=first, stop=last,
                                perf_mode=DR, tile_position=(0, 0),
                            )
                        else:
                            nc.tensor.matmul(
                                ot_ps[0:DK, :],
                                vh[:, p, 2 * kc2 : 2 * kc2 + 2, 0, :],
                                e[:],
                                start=False, stop=False,
                                perf_mode=DR, tile_position=(0, 0),
                            )
                            nc.tensor.matmul(
                                z_ps[0:DK, :],
                                zo_sb[:, :, 0:64],
                                e[:],
                                start=False, stop=False,
                                perf_mode=DR, tile_position=(0, 0),
                            )

                    for kc2 in range(KC // 2):
                        for hh in range(2):
                            h = 2 * p + hh
                            pp = 32 * (h % 4)
                            sc = psA.tile([P, 2 * Q], F32, tag="mm",
                                          name=f"sc_{p}_{kc2}_{hh}")
                            for sub in range(2):
                                kc = 2 * kc2 + sub
                                nc.tensor.matmul(
                                    sc[:, sub * Q : (sub + 1) * Q],
                                    khT32[pp : pp + 32, h // 4, :, kc * P : (kc + 1) * P],
                                    qhT32[pp : pp + 32, h // 4, :, :],
                                    start=True, stop=True,
                                    perf_mode=DR, tile_position=(pp, 0),
                                )
                            e = sexp.tile([P, 2, Q], F8, tag="e",
                                          name=f"e_{p}_{kc2}_{hh}")
                            if (kc2, hh) in dve_slots or (p, kc2, hh) in dve_extra:
                                # int8-Schraudolph exp on the DVE
                                nc.vector.tensor_scalar(
                                    e[:].bitcast(I8), sc[:], A_TR, B_TR,
                                    ALU.mult, ALU.add,
                                )
                            else:
                                nc.scalar.activation(e[:], sc[:], AF.Exp, scale=TEMP_INV)
                            pending.append((e, kc2, hh))
                            if kc2 == 0 and hh == 1:
                                # bank-reset order: the M=128 hh=1 matmul
                                # (start=True) must hit PSUM before hh=0
                                pending[-2], pending[-1] = pending[-1], pending[-2]
                            # interleave next pair's projection work into the
                            # exp-wait gaps
                            take = (len(interleave) - ii) // (8 - (kc2 * 2 + hh)) if interleave else 0
                            for _ in range(take):
                                interleave[ii]()
                                ii += 1
                            # drop the PV lag to 1 near the pair tail so fewer
                            # flushes sit between the last exp and the next
                            # pair's QK matmuls in the in-order PE queue
                            while len(pending) > (2 if kc2 < 2 else 1):
                                flush_one()
                    while ii < len(interleave):
                        interleave[ii]()
                        ii += 1
                    while pending:
                        flush_one()

                    # 1/Z (replicated per-head across partitions by the PE)
                    zb = znorm.tile([P, Q], F32, tag="zb")
                    nc.vector.reciprocal(zb[:], z_ps[:])
                    # fused normalize + PSUM->SBUF copy (fp8 for the DR out-proj)
                    nc.vector.scalar_tensor_tensor(
                        otn[:, p, :], ot_ps[:], 1.0, zb[:], ALU.bypass, ALU.mult
                    )

                # epilogue: deferred into the NEXT repeat's pairs 0-3 (or run
                # inline after the final repeat)
                epi_chains = emit_epilogue_chains(cur)
                if nxt_state is not None:
                    cur = nxt_state
            for ch in epi_chains:
                run_thunks(ch)

    nc.compile()
    return nc


def _get_nc(trivial_ln: bool, repeat: int = 1):
    key = ("nc", trivial_ln, repeat)
    if key not in _CACHE:
        _CACHE[key] = _build(trivial_ln, repeat)
    return _CACHE[key]


def make_in_maps(q, k, v, w_q, w_k, w_v, w_proj, scale, offset):
    q = np.asarray(q, dtype=np.float32)
    k = np.asarray(k, dtype=np.float32)
    v = np.asarray(v, dtype=np.float32)
    scale = np.asarray(scale, dtype=np.float32)
    offset = np.asarray(offset, dtype=np.float32)

    # weights: [H, D, DK] -> [D, H*DK]; w_proj: [D, H*DK] -> [H*DK, D]
    wq2 = np.ascontiguousarray(
        np.transpose(np.asarray(w_q, np.float32), (1, 0, 2)).reshape(D, H * DK)
    ).astype(F8_NP)
    wk2 = np.ascontiguousarray(
        np.transpose(np.asarray(w_k, np.float32), (1, 0, 2)).reshape(D, H * DK)
    ).astype(F8_NP)
    wv2 = np.ascontiguousarray(
        np.transpose(np.asarray(w_v, np.float32), (1, 0, 2)).reshape(D, H * DK)
    ).astype(F8_NP)
    wp2 = np.ascontiguousarray(np.asarray(w_proj, np.float32).T).astype(F8_NP)

    kT_b = [np.ascontiguousarray(k[b].T).astype(F8_NP) for b in range(4)]
    vT_b = [np.ascontiguousarray(v[b].T).astype(F8_NP) for b in range(4)]
    ident = np.eye(P, dtype=BF_NP)

    in_maps = []
    for c in range(8):
        b, qs = c // 2, (c % 2) * Q
        qblk = q[b, qs : qs + Q, :]
        in_maps.append(
            {
                "qT": np.ascontiguousarray(qblk.T).astype(F8_NP),
                "kT": kT_b[b],
                "vT": vT_b[b],
                "wq": wq2,
                "wk": wk2,
                "wv": wv2,
                "wp": wp2,
                "qres": np.ascontiguousarray(qblk).astype(BF_NP),
                "ident": ident,
                "lnsc": scale,
                "lnof": offset,
            }
        )
    return in_maps


def kernel(q, k, v, w_q, w_k, w_v, w_proj, scale, offset):
    scale = np.asarray(scale, dtype=np.float32)
    offset = np.asarray(offset, dtype=np.float32)
    trivial_ln = bool(np.all(scale == 1.0) and np.all(offset == 0.0))
    nc = _get_nc(trivial_ln)
    in_maps = make_in_maps(q, k, v, w_q, w_k, w_v, w_proj, scale, offset)

    res = run_bass_kernel_spmd(nc, in_maps, core_ids=list(range(8)))

    out = np.empty((4, L, D), dtype=np.float32)
    for c in range(8):
        b, qs = c // 2, (c % 2) * Q
        out[b, qs : qs + Q, :] = res.results[c]["out"]
    return out


# revision 27
# speedup vs baseline: 3.4296x; 3.4296x over previous
"""Trainium2 Bass kernel for nn_MultiHeadAttn (16-head attention + out-proj +
residual + layernorm), distributed over 8 NeuronCores.

Sharding: core c handles batch b = c//2 and query rows [512*(c%2), 512*(c%2)+512).
Each core recomputes the full K/V projections for its batch (duplicated between
the two cores of a batch) so there are no collectives; every core is fully
independent and the host just concatenates the 8 output slabs.

All heavy matmuls run fp8e4m3 with DoubleRow (0.5 cycles/row on the PE):
  qhT/khT    = (q|k @ w)          fp8 DR over d_model 128-chunk pairs
  qhT32/khT32: fp8 copies reshuffled via SBUF->SBUF DMA into a [32, 2, *]
               layout so the dk=64 contraction of QK can also run DR
  scoresT    = khT32.T-chunks @ qhT32   fp8 DR, [key_chunk=128, 512] PSUM
  e          = exp(scoresT / 32), fp8: most tiles on ACT (spline exp, fp8
               out); a tunable subset on DVE via the int8-Schraudolph bit
               trick (one tensor_scalar: bits8 = round(8*log2e/32 * s +
               55.66) written through an int8 bitcast of the fp8 tile; max
               rel err ~7.5% vs fp8-quantized true exp's ~5.9%, rms ~3.1%
               vs 2.7%) to offload the ACT bottleneck
  vh_ext     = v @ w_v staged as [128, pair, kc, 3, 64] blocks: block 0 =
               head0, block 1 = zeros (memset once), block 2 = head1; one
               strided-dst DVE copy fills blocks 0 and 2 per pair
  OT/Z       : ALL DoubleRow. head1 writes rows 64-127 through an M=128
               matmul whose lhsT is [zeros(64) | v1(64)] -- the zero half
               accumulates +0 into head0's rows, so the dst partition
               offset stays 0 (walrus s3d3_mm_valid_dst_partition forbids
               DR at offset 64). Per (kc2, hh): one OT + one Z matmul at
               256 PE cycles each vs the old plain-fp8 head1 path's 2048.
               Z uses the same trick with [ones|zeros|ones] lhsT slices;
               accumulation order per PSUM bank starts with the hh=1 M=128
               matmul (start=True resets all 128 rows).
  OTn        = OT * (1/Z)         DVE reciprocal + scalar_tensor_tensor
  out        = OTn.T @ w_projT    fp8 DR over head-pair pairs
  final      = layernorm(out + q_resid): mean/var via bn_stats; rsqrt via
               a quake-style seed (one DVE tensor_scalar in int32 bitcast
               domain: bits = round(C - 0.5*bits(var))) plus one Newton
               step with sqrt((D-1)/D) folded into its constants. No ACT
               Ln/Exp -> the whole kernel uses exactly one ACT table set
               (exp_and_others), eliminating two ~2.7us table reloads per
               iteration.

The attention inner loop interleaves Q/K/V projections for pair p+1 into
pair p's exp-wait gaps; PV/Z matmuls lag their exp by two steps (one near
the pair tail). Cross-repeat software pipelining: input tiles are double-
buffered so the next repeat's DMA loads fire at pair 1, the next repeat's
pair-0 projections hide in pair 7's gaps, and the epilogue is split into
four per-qc chains (out-proj matmuls, bn stats, rsqrt, DVE normalize,
store) drained into pairs 0-3 of the NEXT repeat (otn double-buffered to
decouple the out-proj reads from the next repeat's stt writes). The
normalize stays on the DVE so the in-order ACT queue holds only exps --
an ACT-side normalize waiting on the DVE rsqrt chain would head-of-line
block the following exps.

Numerics: validated in np emulation against the reference: all-fp8 with
40% Schraudolph-exp tiles gives rel err ~2.1e-3 (budget 2e-2); measured
on HW: 2.5e-3.
"""

import sys

sys.path.insert(0, "/opt/trn_rl_repo")

import numpy as np
import ml_dtypes

import concourse.bass as bass
import concourse.mybir as mybir
import concourse.tile as tile
from concourse import bacc
from concourse.bass_utils import run_bass_kernel_spmd

D = 1024          # d_model
H = 16            # heads
DK = 64           # head dim
L = 1024          # seq len (keys)
Q = 512           # query rows per core
P = 128
KC = D // P       # 8 contraction chunks of 128
PAIRS = H // 2    # 8 head pairs
QCN = Q // P      # 4 query chunks
EPS = 1e-5
TEMP_INV = 1.0 / 32.0  # 1/sqrt(d_model)

# int8-Schraudolph exp constants: bits8 = round(A_TR * s + B_TR) viewed fp8e4m3
LOG2E = float(np.log2(np.e))
A_TR = 8.0 * LOG2E / 32.0
B_TR = 55.66
# quake rsqrt seed: bits32 = round(RSQRT_C - 0.5 * bits32(x)); 1 Newton step
RSQRT_C = 1597463174.0           # 0x5f375a86
LN_C = float(np.sqrt((D - 1.0) / D))  # folds unbiased-var factor into Newton

BF = mybir.dt.bfloat16
F8 = mybir.dt.float8e4
F32 = mybir.dt.float32
I8 = mybir.dt.int8
I32 = mybir.dt.int32
AF = mybir.ActivationFunctionType
ALU = mybir.AluOpType
DR = mybir.MatmulPerfMode.DoubleRow
BF_NP = ml_dtypes.bfloat16
F8_NP = ml_dtypes.float8_e4m3

# which (kc2, hh) slots of every pair compute exp on the DVE instead of ACT,
# plus extra per-pair slots for fine-grained ACT/DVE balance
DVE_EXP_SLOTS = frozenset({(1, 1)})
DVE_EXP_EXTRA = frozenset({(0, 3, 1), (4, 3, 1)})

_CACHE: dict = {}


def _build(trivial_ln: bool, repeat: int = 1, dve_slots=DVE_EXP_SLOTS,
           dve_extra=DVE_EXP_EXTRA):
    nc = bacc.Bacc(None, target_bir_lowering=False)

    qT = nc.dram_tensor("qT", [D, Q], F8, kind="ExternalInput")
    kT = nc.dram_tensor("kT", [D, L], F8, kind="ExternalInput")
    vT = nc.dram_tensor("vT", [D, L], F8, kind="ExternalInput")
    wq = nc.dram_tensor("wq", [D, H * DK], F8, kind="ExternalInput")
    wk = nc.dram_tensor("wk", [D, H * DK], F8, kind="ExternalInput")
    wv = nc.dram_tensor("wv", [D, H * DK], F8, kind="ExternalInput")
    wp = nc.dram_tensor("wp", [H * DK, D], F8, kind="ExternalInput")
    qres = nc.dram_tensor("qres", [Q, D], BF, kind="ExternalInput")
    ident = nc.dram_tensor("ident", [P, P], BF, kind="ExternalInput")
    lnsc = nc.dram_tensor("lnsc", [D], F32, kind="ExternalInput")
    lnof = nc.dram_tensor("lnof", [D], F32, kind="ExternalInput")
    out = nc.dram_tensor("out", [Q, D], F32, kind="ExternalOutput")

    with tile.TileContext(nc) as tc:
        with (
            tc.tile_pool(name="consts", bufs=1) as consts,
            tc.tile_pool(name="sexp", bufs=12) as sexp,
            tc.tile_pool(name="znorm", bufs=3) as znorm,
            tc.tile_pool(name="lnp", bufs=3) as lnp,
            tc.tile_pool(name="psA", bufs=3, space="PSUM") as psA,
            tc.tile_pool(name="psOT", bufs=1, space="PSUM") as psOT,
            tc.tile_pool(name="psZ", bufs=1, space="PSUM") as psZ,
            # psA 3x[128,1024]f32 = 6 banks; ot/z 1 bank each -> 8 total
        ):
            # -------- constants shared across repeats --------
            # Z lhsT [128, 2, 192]: cols 0:64 ones (z0, M=64), 64:128 zeros +
            # 128:192 ones (z1 via [zeros|ones] M=128 at dst offset 0)
            zo_sb = consts.tile([P, 2, 192], F8, tag="zo")
            nc.vector.memset(zo_sb[:], 1.0)
            nc.vector.memset(zo_sb[:, :, 64:128], 0.0)
            ident_sb = consts.tile([P, P], BF, tag="ident")
            nc.sync.dma_start(ident_sb[:], ident.ap())
            if not trivial_ln:
                lnsc_b = consts.tile([P, D], F32, tag="lnsc")
                nc.gpsimd.dma_start(
                    lnsc_b[:],
                    bass.AP(tensor=lnsc.ap().tensor, offset=0, ap=[[0, P], [1, D]]),
                )
                lnof_b = consts.tile([P, D], F32, tag="lnof")
                nc.gpsimd.dma_start(
                    lnof_b[:],
                    bass.AP(tensor=lnof.ap().tensor, offset=0, ap=[[0, P], [1, D]]),
                )
            # magic-seed constant tile for the DVE epilogue (int32 bits)
            rsq_c = lnp.tile([P, 2], F32, tag="rsqc")
            nc.vector.memset(rsq_c[:].bitcast(I32), 0x5F375A86)

            # per-repeat tile state. Input tiles are DOUBLE-BUFFERED (tag
            # alternates with the repeat parity) so the next repeat's DMA
            # loads can issue early in the current repeat with no WAR wait;
            # rep-local intermediates (qhT8/khT32/vh/otn/...) stay
            # single-buffered -- cross-repeat overlap only touches disjoint
            # sub-tile regions, which the framework tracks.
            vhz_zeroed = [False]

            def emit_loads(parity):
                b = parity & 1
                t = {}
                t["qT_sb"] = consts.tile([P, KC, Q], F8, tag=f"qT{b}", name="qT_sb")
                nc.sync.dma_start(t["qT_sb"][:], qT.ap().rearrange("(c p) q -> p c q", p=P))
                t["wq_sb"] = consts.tile([P, KC, H * DK], F8, tag=f"wq{b}", name="wq_sb")
                nc.sync.dma_start(t["wq_sb"][:], wq.ap().rearrange("(c p) m -> p c m", p=P))
                t["kT_sb"] = consts.tile([P, KC, L], F8, tag=f"kT{b}", name="kT_sb")
                nc.sync.dma_start(t["kT_sb"][:], kT.ap().rearrange("(c p) q -> p c q", p=P))
                t["wk_sb"] = consts.tile([P, KC, H * DK], F8, tag=f"wk{b}", name="wk_sb")
                nc.sync.dma_start(t["wk_sb"][:], wk.ap().rearrange("(c p) m -> p c m", p=P))
                t["vT_sb"] = consts.tile([P, KC, L], F8, tag=f"vT{b}", name="vT_sb")
                nc.gpsimd.dma_start(
                    t["vT_sb"][:], vT.ap().rearrange("(c p) q -> p c q", p=P))
                t["wv_sb"] = consts.tile([P, KC, H * DK], F8, tag=f"wv{b}", name="wv_sb")
                nc.gpsimd.dma_start(
                    t["wv_sb"][:], wv.ap().rearrange("(c p) m -> p c m", p=P))
                t["wp_sb"] = consts.tile([P, PAIRS, D], F8, tag=f"wp{b}", name="wp_sb")
                nc.gpsimd.dma_start(
                    t["wp_sb"][:], wp.ap().rearrange("(c p) m -> p c m", p=P))
                t["qres_sb"] = consts.tile([P, QCN, D], BF, tag=f"qres{b}", name="qres_sb")
                nc.gpsimd.dma_start(
                    t["qres_sb"][:], qres.ap().rearrange("(c p) d -> p c d", p=P))

                t["qhT8"] = consts.tile([P, PAIRS, Q], F8, tag="qhT8", name="qhT8")
                t["khT8"] = consts.tile([P, PAIRS, L], F8, tag="khT8", name="khT8")
                # [32,2] DR layouts: head h lives on partitions 32*(h%4)+[0,32),
                # free dims (h//4, j, seq) with dk = 32*j + p
                t["qhT32"] = consts.tile([P, 4, 2, Q], F8, tag="qhT32", name="qhT32")
                t["khT32"] = consts.tile([P, 4, 2, L], F8, tag="khT32", name="khT32")
                # vh_ext per pair: [P, pair, kc, 3 blocks, 64]: block 0 head0
                # v, block 1 zeros (for the M=128 [zeros|v1] PV trick), block
                # 2 head1 v. Zeros memset once (tag-stable buffer); the
                # conversion writes blocks 0 and 2 in ONE strided-dst copy.
                t["vh"] = consts.tile([P, PAIRS, KC, 3, 64], F8, tag="vh", name="vh")
                # otn double-buffered: the epilogue of rep N reads otn while
                # rep N+1's stt writes the other buffer
                t["otn"] = consts.tile([P, PAIRS, Q], F8, tag=f"otn{b}", name="otn")
                if not vhz_zeroed[0]:
                    nc.vector.memset(t["vh"][:, :, :, 1, :], 0.0)
                    vhz_zeroed[0] = True
                return t

            # ---------------- per-pair projection emitters ----------------
            def emit_qkproj(m, cur):
                """Q+K projections for pair m (against state `cur`) as thunks."""
                st = {}

                def qmm(c2):
                    if "q" not in st:
                        st["q"] = psA.tile([P, 2 * Q], F32, tag="mm", name=f"psq_{m}")
                    nc.tensor.matmul(
                        st["q"][:, :Q],
                        cur["wq_sb"][:, 2 * c2 : 2 * c2 + 2, m * P : (m + 1) * P],
                        cur["qT_sb"][:, 2 * c2 : 2 * c2 + 2, :],
                        start=(c2 == 0), stop=(c2 == KC // 2 - 1),
                        perf_mode=DR,
                    )

                def kmm(half, c2):
                    if "k" not in st:
                        st["k"] = psA.tile([P, L], F32, tag="mm", name=f"psk_{m}")
                    nc.tensor.matmul(
                        st["k"][:, half * 512 : (half + 1) * 512],
                        cur["wk_sb"][:, 2 * c2 : 2 * c2 + 2, m * P : (m + 1) * P],
                        cur["kT_sb"][:, 2 * c2 : 2 * c2 + 2, half * 512 : (half + 1) * 512],
                        start=(c2 == 0), stop=(c2 == KC // 2 - 1),
                        perf_mode=DR,
                    )

                def shuffle(is_q, hh, j):
                    # split across the Pool (SWDGE) and SP (HWDGE) rings
                    h = 2 * m + hh
                    dst32 = cur["qhT32"] if is_q else cur["khT32"]
                    src8 = cur["qhT8"] if is_q else cur["khT8"]
                    eng = nc.gpsimd if is_q else nc.sync
                    eng.dma_start(
                        dst32[32 * (h % 4) : 32 * (h % 4) + 32, h // 4, j, :],
                        src8[hh * DK + 32 * j : hh * DK + 32 * j + 32, m, :],
                    )

                th = []
                for c2 in range(KC // 2):
                    th.append(lambda c2=c2: qmm(c2))
                th.append(lambda: nc.vector.tensor_copy(
                    cur["qhT8"][:, m, :], st["q"][:, :Q]))
                for hh in range(2):
                    for j in range(2):
                        th.append(lambda hh=hh, j=j: shuffle(True, hh, j))
                for half in range(2):
                    for c2 in range(KC // 2):
                        th.append(lambda half=half, c2=c2: kmm(half, c2))
                th.append(lambda: nc.vector.tensor_copy(
                    cur["khT8"][:, m, :], st["k"][:]))
                for hh in range(2):
                    for j in range(2):
                        th.append(lambda hh=hh, j=j: shuffle(False, hh, j))
                return th

            def emit_vproj(m, cur):
                """V projection for pair m into the vh_ext [.., 192] layout."""
                st = {}

                def vmm(kc, c2):
                    if "v" not in st:
                        st["v"] = psA.tile([P, KC, P], F32, tag="mm", name=f"psv_{m}")
                    nc.tensor.matmul(
                        st["v"][:, kc, :],
                        cur["vT_sb"][:, 2 * c2 : 2 * c2 + 2, kc * P : (kc + 1) * P],
                        cur["wv_sb"][:, 2 * c2 : 2 * c2 + 2, m * P : (m + 1) * P],
                        start=(c2 == 0), stop=(c2 == KC // 2 - 1),
                        perf_mode=DR,
                    )

                th = []
                for kc in range(KC):
                    for c2 in range(KC // 2):
                        th.append(lambda kc=kc, c2=c2: vmm(kc, c2))
                # one strided-dst copy fills head0 (block 0) and head1
                # (block 2), skipping the zeros block
                th.append(lambda: nc.vector.tensor_copy(
                    cur["vh"][:, m, :, 0:3:2, :],
                    st["v"][:].rearrange("p k (g c) -> p k g c", g=2)))
                return th

            def run_thunks(th):
                for t in th:
                    t()

            def emit_epilogue_chains(cur):
                """Output projection + residual + layernorm for the rep whose
                state is `cur`, as FOUR independent per-qc thunk chains. Each
                chain is drained into one pair of the NEXT repeat's attention
                (or run inline after the last repeat): out-proj matmuls fill
                PE gaps, bn/rsqrt the DVE, and the final normalize runs on
                ACT (Identity with per-partition scale/bias) between exps."""
                otn_t, wp_t, qres_t = cur["otn"], cur["wp_sb"], cur["qres_sb"]

                def make_chain(qc):
                    st = {}

                    def fpmm(half, p2, qc=qc, st=st):
                        if "fp" not in st:
                            st["fp"] = psA.tile([P, D], F32, tag="mm", name=f"fp_{qc}")
                        nc.tensor.matmul(
                            st["fp"][:, half * 512 : (half + 1) * 512],
                            otn_t[:, 2 * p2 : 2 * p2 + 2, qc * P : (qc + 1) * P],
                            wp_t[:, 2 * p2 : 2 * p2 + 2, half * 512 : (half + 1) * 512],
                            start=(p2 == 0), stop=False,
                            perf_mode=DR,
                        )

                    def identmm(half, qc=qc, st=st):
                        # residual folded into the accumulation
                        nc.tensor.matmul(
                            st["fp"][:, half * 512 : (half + 1) * 512],
                            ident_sb[:],
                            qres_t[:, qc, half * 512 : (half + 1) * 512],
                            start=False, stop=True,
                        )

                    def bn(j, st=st):
                        if "stats" not in st:
                            st["stats"] = lnp.tile([P, 2, 6], F32, tag="stats", name="stats")
                        nc.vector.bn_stats(st["stats"][:, j, :],
                                           st["fp"][:, j * 512 : (j + 1) * 512])

                    def aggr(st=st):
                        st["mv"] = lnp.tile([P, 2], F32, tag="mv", name="mv")
                        nc.vector.bn_aggr(st["mv"][:], st["stats"][:])

                    def rchain(st=st):
                        # rinv = LN_C/sqrt(var): quake seed + one Newton step,
                        # then negms = -mean*rinv, all tiny [P,1] DVE ops
                        mv = st["mv"]
                        y0 = lnp.tile([P, 1], F32, tag="y0")
                        nc.vector.scalar_tensor_tensor(
                            y0[:].bitcast(I32), mv[:, 1:2].bitcast(I32), -0.5,
                            rsq_c[:, 0:1].bitcast(I32), ALU.mult, ALU.add,
                        )
                        ta = lnp.tile([P, 1], F32, tag="ta")
                        nc.vector.tensor_mul(ta[:], y0[:], mv[:, 1:2])
                        t2 = lnp.tile([P, 1], F32, tag="t2")
                        nc.vector.tensor_mul(t2[:], ta[:], y0[:])
                        t3 = lnp.tile([P, 1], F32, tag="t3")
                        nc.vector.tensor_scalar(
                            t3[:], t2[:], -0.5 * LN_C, 1.5 * LN_C, ALU.mult, ALU.add
                        )
                        rinv = lnp.tile([P, 1], F32, tag="rinv")
                        nc.vector.tensor_mul(rinv[:], y0[:], t3[:])
                        st["rinv"] = rinv

                    def final(qc=qc, st=st):
                        # normalize on the DVE: keeps the ACT queue pure-exp
                        # (an ACT-side ts would head-of-line block later exps
                        # while waiting on the DVE rsqrt chain)
                        o_sb = lnp.tile([P, D], F32, tag="o", name=f"o_{qc}")
                        nc.vector.tensor_scalar(
                            o_sb[:], st["fp"][:], st["mv"][:, 0:1],
                            st["rinv"][:, 0:1], ALU.subtract, ALU.mult,
                        )
                        if not trivial_ln:
                            nc.vector.tensor_mul(o_sb[:], o_sb[:], lnsc_b[:])
                            nc.vector.tensor_add(o_sb[:], o_sb[:], lnof_b[:])
                        nc.sync.dma_start(out.ap()[qc * P : (qc + 1) * P, :], o_sb[:])

                    th = []
                    for half in range(2):
                        for p2 in range(PAIRS // 2):
                            th.append(lambda half=half, p2=p2: fpmm(half, p2))
                        th.append(lambda half=half: identmm(half))
                    th.append(lambda: bn(0))
                    th.append(lambda: bn(1))
                    th.append(aggr)
                    th.append(rchain)
                    th.append(final)
                    return th

                return [make_chain(qc) for qc in range(QCN)]

            cur = emit_loads(0)
            run_thunks(emit_qkproj(0, cur))
            run_thunks(emit_vproj(0, cur))
            epi_chains: list = []  # previous rep's epilogue, one chain per qc
            for _rep in range(repeat):
                nxt_state = None
                qhT32, khT32 = cur["qhT32"], cur["khT32"]
                vh, otn = cur["vh"], cur["otn"]

                # ---------------- attention (per head pair), pipelined --------
                for p in range(PAIRS):
                    if p == 1 and _rep + 1 < repeat:
                        # kick off the NEXT repeat's input DMAs now -- the
                        # double-buffered tiles have no WAR, so the loads
                        # complete long before pair-7's prologue thunks
                        # consume them
                        nxt_state = emit_loads(_rep + 1)
                    if p + 1 < PAIRS:
                        interleave = emit_qkproj(p + 1, cur) + emit_vproj(p + 1, cur)
                    elif nxt_state is not None:
                        # hoist the NEXT repeat's pair-0 projections into the
                        # LAST pair's exp-wait gaps, so the next attention
                        # phase is ready the moment this repeat ends
                        interleave = (emit_qkproj(0, nxt_state)
                                      + emit_vproj(0, nxt_state))
                    else:
                        interleave = []
                    if p < len(epi_chains):
                        # drain the PREVIOUS repeat's epilogue chain for qc=p
                        # into this pair's exp-wait gaps
                        interleave = epi_chains[p] + interleave
                    ii = 0
                    ot_ps = psOT.tile([P, Q], F32, tag="ot")
                    z_ps = psZ.tile([P, Q], F32, tag="z")
                    pending = []  # lagged PV/Z emissions: (e_tile, kc2, hh)

                    def flush_one():
                        e, kc2, hh = pending.pop(0)
                        first = kc2 == 0
                        last = kc2 == KC // 2 - 1
                        if hh == 1:
                            # M=128 [zeros(64)|v1(64)] lhsT: head1 lands on
                            # rows 64-127 while rows 0-63 accumulate +0, so
                            # the DR dst offset stays 0. start=True on the
                            # kc2==0 instance resets the whole bank (it is
                            # flushed first; see the pending swap below).
                            nc.tensor.matmul(
                                ot_ps[:, :],
                                vh[:, p, 2 * kc2 : 2 * kc2 + 2, 1:3, :],
                                e[:],
                                start=first, stop=last,
                                perf_mode=DR, tile_position=(0, 0),
                            )
                            nc.tensor.matmul(
                                z_ps[:, :],
                                zo_sb[:, :, 64:192],
                                e[:],
                                start=first, stop=last,
                                perf_mode=DR, tile_position=(0, 0),
                            )
                        else:
                            nc.tensor.matmul(
                                ot_ps[0:DK, :],
                                vh[:, p, 2 * kc2 : 2 * kc2 + 2, 0, :],
                                e[:],
                                start=False, stop=False,
                                perf_mode=DR, tile_position=(0, 0),
                            )
                            nc.tensor.matmul(
                                z_ps[0:DK, :],
                                zo_sb[:, :, 0:64],
                                e[:],
                                start=False, stop=False,
                                perf_mode=DR, tile_position=(0, 0),
                            )

                    for kc2 in range(KC // 2):
                        for hh in range(2):
                            h = 2 * p + hh
                            pp = 32 * (h % 4)
                            sc = psA.tile([P, 2 * Q], F32, tag="mm",
                                          name=f"sc_{p}_{kc2}_{hh}")
                            for sub in range(2):
                                kc = 2 * kc2 + sub
                                nc.tensor.matmul(
                                    sc[:, sub * Q : (sub + 1) * Q],
                                    khT32[pp : pp + 32, h // 4, :, kc * P : (kc + 1) * P],
                                    qhT32[pp : pp + 32, h // 4, :, :],
                                    start=True, stop=True,
                                    perf_mode=DR, tile_position=(pp, 0),
                                )
                            e = sexp.tile([P, 2, Q], F8, tag="e",
                                          name=f"e_{p}_{kc2}_{hh}")
                            if (kc2, hh) in dve_slots or (p, kc2, hh) in dve_extra:
                                # int8-Schraudolph exp on the DVE
                                nc.vector.tensor_scalar(
                                    e[:].bitcast(I8), sc[:], A_TR, B_TR,
                                    ALU.mult, ALU.add,
                                )
                            else:
                                nc.scalar.activation(e[:], sc[:], AF.Exp, scale=TEMP_INV)
                            pending.append((e, kc2, hh))
                            if kc2 == 0 and hh == 1:
                                # bank-reset order: the M=128 hh=1 matmul
                                # (start=True) must hit PSUM before hh=0
                                pending[-2], pending[-1] = pending[-1], pending[-2]
                            # interleave next pair's projection work into the
                            # exp-wait gaps
                            take = (len(interleave) - ii) // (8 - (kc2 * 2 + hh)) if interleave else 0
                            for _ in range(take):
                                interleave[ii]()
                                ii += 1
                            # drop the PV lag to 1 near the pair tail so fewer
                            # flushes sit between the last exp and the next
                            # pair's QK matmuls in the in-order PE queue
                            while len(pending) > (2 if kc2 < 2 else 1):
                                flush_one()
                    while ii < len(interleave):
                        interleave[ii]()
                        ii += 1
                    while pending:
                        flush_one()

                    # 1/Z (replicated per-head across partitions by the PE)
                    zb = znorm.tile([P, Q], F32, tag="zb")
                    nc.vector.reciprocal(zb[:], z_ps[:])
                    # fused normalize + PSUM->SBUF copy (fp8 for the DR out-proj)
                    nc.vector.scalar_tensor_tensor(
                        otn[:, p, :], ot_ps[:], 1.0, zb[:], ALU.bypass, ALU.mult
                    )

                # epilogue: deferred into the NEXT repeat's pairs 0-3 (or run
                # inline after the final repeat)
                epi_chains = emit_epilogue_chains(cur)
                if nxt_state is not None:
                    cur = nxt_state
            for ch in epi_chains:
                run_thunks(ch)

    nc.compile()
    return nc


def _get_nc(trivial_ln: bool, repeat: int = 1):
    key = ("nc", trivial_ln, repeat)
    if key not in _CACHE:
        _CACHE[key] = _build(trivial_ln, repeat)
    return _CACHE[key]


def make_in_maps(q, k, v, w_q, w_k, w_v, w_proj, scale, offset):
    q = np.asarray(q, dtype=np.float32)
    k = np.asarray(k, dtype=np.float32)
    v = np.asarray(v, dtype=np.float32)
    scale = np.asarray(scale, dtype=np.float32)
    offset = np.asarray(offset, dtype=np.float32)

    # weights: [H, D, DK] -> [D, H*DK]; w_proj: [D, H*DK] -> [H*DK, D]
    wq2 = np.ascontiguousarray(
        np.transpose(np.asarray(w_q, np.float32), (1, 0, 2)).reshape(D, H * DK)
    ).astype(F8_NP)
    wk2 = np.ascontiguousarray(
        np.transpose(np.asarray(w_k, np.float32), (1, 0, 2)).reshape(D, H * DK)
    ).astype(F8_NP)
    wv2 = np.ascontiguousarray(
        np.transpose(np.asarray(w_v, np.float32), (1, 0, 2)).reshape(D, H * DK)
    ).astype(F8_NP)
    wp2 = np.ascontiguousarray(np.asarray(w_proj, np.float32).T).astype(F8_NP)

    kT_b = [np.ascontiguousarray(k[b].T).astype(F8_NP) for b in range(4)]
    vT_b = [np.ascontiguousarray(v[b].T).astype(F8_NP) for b in range(4)]
    ident = np.eye(P, dtype=BF_NP)

    in_maps = []
    for c in range(8):
        b, qs = c // 2, (c % 2) * Q
        qblk = q[b, qs : qs + Q, :]
        in_maps.append(
            {
                "qT": np.ascontiguousarray(qblk.T).astype(F8_NP),
                "kT": kT_b[b],
                "vT": vT_b[b],
                "wq": wq2,
                "wk": wk2,
                "wv": wv2,
                "wp": wp2,
                "qres": np.ascontiguousarray(qblk).astype(BF_NP),
                "ident": ident,
                "lnsc": scale,
                "lnof": offset,
            }
        )
    return in_maps


def kernel(q, k, v, w_q, w_k, w_v, w_proj, scale, offset):
    scale = np.asarray(scale, dtype=np.float32)
    offset = np.asarray(offset, dtype=np.float32)
    trivial_ln = bool(np.all(scale == 1.0) and np.all(offset == 0.0))
    nc = _get_nc(trivial_ln)
    in_maps = make_in_maps(q, k, v, w_q, w_k, w_v, w_proj, scale, offset)

    res = run_bass_kernel_spmd(nc, in_maps, core_ids=list(range(8)))

    out = np.empty((4, L, D), dtype=np.float32)
    for c in range(8):
        b, qs = c // 2, (c % 2) * Q
        out[b, qs : qs + Q, :] = res.results[c]["out"]
    return out
